# revision 1
# baseline (speedup 1.0000x reference)
"""MoE (AriaExperts) Trainium2 kernel — expert parallelism across 8 NeuronCores.

Strategy:
  - Host: top-2 routing + softmax over [2048, 8] logits (tiny), build the
    per-expert token batches (the "all-to-all" is realized at input
    distribution time), and the weighted scatter-add combine at the end.
  - Device (SPMD, 1 expert per core): dense GEMM chain in transposed
    activation layout so both matmuls consume the expert weights directly
    as the stationary (lhsT) operand with zero on-device transposes:
        H^T  = W1^T-tiles @ X^T      [2*INTER, C]
        actT = silu(projT) * gateT   [INTER, C]
        outT = W2-tiles   @ actT     [HIDDEN, C]
    bf16 matmuls with f32 PSUM accumulation (1 cycle/row vs 4 for f32).

  Engine budget (per core, C=512): 384 real matmuls x 216 ns = 83 us on
  the PE at the warm 2.4 GHz clock; total input DMA 13.6 MB ~ 35-40 us,
  fully overlapped. Trace-driven layout decisions:
    - PSUM pool = 8 x 1-bank [128,512] slots so 4 FC1 proj/gate pairs can
      be in flight; SwiGLU readout lags PE by several pairs without
      blocking PSUM recycling (2-bank slots measured a 4.2 us PE stall +
      HAM re-throttle when the first silu ran late).
    - ACT (scalar) engine runs ONLY the 16 silus: FC2 PSUM->SBUF copies
      live on DVE, and scalar issues just 5 early DMA triggers whose
      completion-semaphore lanes are provably fresh (a trigger re-using a
      lane waits for the prior DMA on that lane and, being FIFO, blocks
      every silu behind it — measured 11 us of silu delay).
    - A dummy 8-element silu right after the triggers forces the SILU
      ACT_TABLE_LOAD (1.3 us) during the initial DMA window instead of
      lazily in front of the first real silu.
    - Input bytes split across both HWDGE rings (sync+scalar) in
      consumption order, xt first, w1 head slots individually, w2 halves
      last; measured combined delivery ~400 GB/s.
    - PE warmup: a short burst of matmuls on a memset tile flips the HAM
      clock-gate (1.2 -> 2.4 GHz needs ~3.4 us of sustained busy) while
      the first inputs stream in.
"""

import time

import numpy as np
import ml_dtypes

import concourse.bass as bass
import concourse.bacc as bacc
import concourse.mybir as mybir
import concourse.tile as tile
from concourse.bass_utils import run_bass_kernel_spmd

NUM_TOKENS = 2048
HIDDEN = 1024
INTER = 2048
NUM_EXPERTS = 8
TOPK = 2
NCORES = 8
P = 128
KT1 = HIDDEN // P         # 8  k-tiles (FC1 contraction)
MT1 = 2 * INTER // P      # 32 m-tiles (FC1 output rows = proj+gate)
MT1H = INTER // P         # 16 proj/gate pair count
KT2 = INTER // P          # 16 k-tiles (FC2 contraction)
MT2 = HIDDEN // P         # 8  m-tiles (FC2 output rows)

BF16 = mybir.dt.bfloat16
F32 = mybir.dt.float32
np_bf16 = ml_dtypes.bfloat16

# [0, 16, 1, 17, ...] — interleave proj/gate m-tiles into adjacent pairs
_W1_ORDER = np.arange(MT1).reshape(2, MT1H).T.reshape(-1)

NWARM = 14  # warmup matmuls: HAM flip (~3.4 us busy) + bridge until pair-0
            # data lands (~13.5 us with the 3-stream head; rings ramp
            # slowly over the first ~5 us). Late warmups run at the warm
            # clock, so the tail of the warmup self-compresses; 14 keeps the
            # worst-case (all-cold-phase) handoff just ahead of data-ready.

_graph_cache: dict = {}


def _build(NCH: int, CH: int) -> bass.Bass:
    """Per-core Bass graph for capacity C_pad = NCH * CH (CH <= 512)."""
    nc = bacc.Bacc("TRN2", target_bir_lowering=False, debug=False)

    xt_d = nc.declare_dram_parameter("xt", [P, KT1, NCH, CH], BF16, isOutput=False)
    w1_d = nc.declare_dram_parameter("w1", [P, MT1, KT1, P], BF16, isOutput=False)
    w2_d = nc.declare_dram_parameter("w2", [P, MT2, KT2, P], BF16, isOutput=False)
    # bf16 output: halves the output DMA on the kernel tail; the host-side
    # combine upcasts to f32 (adds ~0.2% rounding on top of the ~0.4% bf16
    # matmul error — well within the 2e-2 gate).
    out_d = nc.declare_dram_parameter("out", [MT2, NCH, P, CH], BF16, isOutput=True)

    with tile.TileContext(nc) as tc:
        with (
            tc.tile_pool(name="weights", bufs=1) as wpool,
            tc.tile_pool(name="xin", bufs=1) as xpool,
            tc.tile_pool(name="actp", bufs=2) as apool,
            tc.tile_pool(name="tmp", bufs=4) as tpool,
            tc.tile_pool(name="osb", bufs=4) as opool,
            tc.tile_pool(name="psum", bufs=8, space="PSUM") as pspool,
        ):
            xt = xpool.tile([P, KT1, NCH, CH], BF16, tag="xt")
            w1 = wpool.tile([P, MT1, KT1, P], BF16, tag="w1")
            w2 = wpool.tile([P, MT2, KT2, P], BF16, tag="w2")
            dummy = xpool.tile([P, 640], BF16, tag="dummy")

            # Memset on DVE (idle at boot): GpSimd's queue must lead with its
            # SWDGE xt trigger, and the warmup matmuls (which wait on this
            # memset) start ~0.5 us earlier off the DVE than behind GpSimd.
            nc.vector.memset(dummy[:], 0.0)

            # PE warmup on the memset tile while inputs stream in.
            warm_ps = pspool.tile([P, 512], F32, tag="ps", name="warmps")
            for _ in range(NWARM):
                nc.tensor.matmul(
                    warm_ps[:, :], dummy[:, :128], dummy[:, 128:640],
                    start=True, stop=True,
                )

            # Force the SILU activation-table load (~1.3 us, non-blocking)
            # right at engine boot, overlapped with the first input DMAs,
            # instead of lazily in front of the first real silu.
            tdum = tpool.tile([P, 8], F32, tag="tmp", name="tdum")
            nc.scalar.activation(
                tdum[:], dummy[:, :8], mybir.ActivationFunctionType.Silu
            )

            # ---- input DMA triggers ----
            # Chunky head: 128-256 KB transfers measured only ~50-200 GB/s
            # effective on the rings (per-transfer descriptor + completion
            # overhead), so xt goes as two 0.5 MB halves (kt 0-3 on sync,
            # kt 4-7 on scalar — the scheduler interleaves proj/gate per
            # k-tile, so both w1 slots 0 and 1 are needed right at T0 and
            # follow their ring's xt half immediately). Emission alternates
            # sync/scalar to spread the 8 round-robin completion-sem lanes.
            # Chunky head (measured optimum): fine-grained transfers lose to
            # per-transfer ring overhead — the rings deliver only
            # ~100-150 GB/s for the first ~5 us whatever the chunking, so
            # the minimum to start (xt + slots 0/1, ~0.78 MB per ring) lands
            # ~14.5 us and big chunks get there with the fewest stalls.
            # Third stream: GpSimd/SWDGE carries the middle xt k-tiles in
            # parallel with both HWDGE rings during the slow early phase.
            # s0/s1 lead their rings (needed at T0 + 0/0.2 us); xt k-tiles
            # ordered so each lands before its consumption slot.
            nc.sync.dma_start(out=w1[:, 0:1], in_=w1_d[:, 0:1])
            nc.scalar.dma_start(out=w1[:, 1:2], in_=w1_d[:, 1:2])
            nc.gpsimd.dma_start(out=xt[:, 3:5, 0], in_=xt_d[:, 3:5, 0])
            nc.sync.dma_start(out=xt[:, 0:3, 0], in_=xt_d[:, 0:3, 0])
            nc.scalar.dma_start(out=xt[:, 5:8, 0], in_=xt_d[:, 5:8, 0])
            nc.sync.dma_start(out=w1[:, 2:3], in_=w1_d[:, 2:3])
            nc.scalar.dma_start(out=w1[:, 3:4], in_=w1_d[:, 3:4])
            nc.sync.dma_start(out=w1[:, 4:6], in_=w1_d[:, 4:6])
            nc.scalar.dma_start(out=w1[:, 6:8], in_=w1_d[:, 6:8])
            nc.sync.dma_start(out=w1[:, 8:10], in_=w1_d[:, 8:10])
            nc.scalar.dma_start(out=w1[:, 10:12], in_=w1_d[:, 10:12])
            nc.sync.dma_start(out=w1[:, 12:16], in_=w1_d[:, 12:16])
            nc.scalar.dma_start(out=w1[:, 16:20], in_=w1_d[:, 16:20])
            nc.sync.dma_start(out=w1[:, 20:26], in_=w1_d[:, 20:26])
            nc.scalar.dma_start(out=w2[:, MT2 // 2 :], in_=w2_d[:, MT2 // 2 :])
            nc.sync.dma_start(out=w1[:, 26:32], in_=w1_d[:, 26:32])
            nc.sync.dma_start(out=w2[:, : MT2 // 2], in_=w2_d[:, : MT2 // 2])
            for ci in range(1, NCH):
                nc.sync.dma_start(out=xt[:, :, ci], in_=xt_d[:, :, ci])

            for ci in range(NCH):
                # ---- FC1 (proj/gate pair per iteration) + SwiGLU ----
                act = apool.tile([P, KT2, CH], BF16, tag="act", name=f"act{ci}")
                for mt in range(MT1H):
                    ps_p = pspool.tile([P, 512], F32, tag="ps", name=f"psp{ci}_{mt}")
                    ps_g = pspool.tile([P, 512], F32, tag="ps", name=f"psg{ci}_{mt}")
                    for ps, pg in ((ps_p, 0), (ps_g, 1)):
                        for kt in range(KT1):
                            nc.tensor.matmul(
                                ps[:, :CH],
                                w1[:, 2 * mt + pg, kt, :],
                                xt[:, kt, ci, :],
                                start=(kt == 0),
                                stop=(kt == KT1 - 1),
                            )
                    tmp = tpool.tile([P, CH], F32, tag="tmp", name=f"tmp{ci}_{mt}")
                    nc.scalar.activation(
                        tmp[:], ps_p[:, :CH], mybir.ActivationFunctionType.Silu
                    )
                    nc.vector.tensor_mul(act[:, mt], tmp[:], ps_g[:, :CH])

                # ---- FC2 ----
                # Output copies live on DVE and output DMA triggers on the
                # scalar ring (idle during FC2; sync still drains w2/xt).
                for m2 in range(MT2):
                    ps_o = pspool.tile([P, 512], F32, tag="ps", name=f"pso{ci}_{m2}")
                    o_sb = opool.tile([P, CH], BF16, tag="o", name=f"osb{ci}_{m2}")
                    if ci == NCH - 1 and m2 == MT2 - 1:
                        # Final m-tile: two independent half-column chains so
                        # the first half drains (copy+DMA) while the PE runs
                        # the second half — halves the post-last-matmul tail.
                        # Separate PSUM tiles: tile-granular WAR tracking
                        # would otherwise stall chain B behind chain A's copy.
                        ps_b = pspool.tile([P, 512], F32, tag="ps", name="psoB")
                        h = CH // 2
                        for ps, (c0, c1) in ((ps_o, (0, h)), (ps_b, (h, CH))):
                            for kt2 in range(KT2):
                                nc.tensor.matmul(
                                    ps[:, c0:c1],
                                    w2[:, m2, kt2, :],
                                    act[:, kt2, c0:c1],
                                    start=(kt2 == 0),
                                    stop=(kt2 == KT2 - 1),
                                )
                            nc.vector.tensor_scalar_mul(
                                o_sb[:, c0:c1], ps[:, c0:c1], 1.0
                            )
                            nc.scalar.dma_start(
                                out=out_d[m2, ci, :, c0:c1], in_=o_sb[:, c0:c1]
                            )
                    else:
                        for kt2 in range(KT2):
                            nc.tensor.matmul(
                                ps_o[:, :CH],
                                w2[:, m2, kt2, :],
                                act[:, kt2, :],
                                start=(kt2 == 0),
                                stop=(kt2 == KT2 - 1),
                            )
                        nc.vector.tensor_scalar_mul(o_sb[:], ps_o[:, :CH], 1.0)
                        nc.scalar.dma_start(out=out_d[m2, ci], in_=o_sb[:])

    nc.compile()
    return nc


def _get_graph(NCH: int, CH: int) -> bass.Bass:
    key = (NCH, CH)
    if key not in _graph_cache:
        _graph_cache[key] = _build(NCH, CH)
    return _graph_cache[key]


def _route(router_logits: np.ndarray):
    """Top-2 + softmax, exactly matching jax.lax.top_k tie-breaking."""
    idx = np.argsort(-router_logits, axis=-1, kind="stable")[:, :TOPK]
    tl = np.take_along_axis(router_logits, idx, axis=-1)
    ex = np.exp(tl - tl.max(-1, keepdims=True))
    sc = (ex / ex.sum(-1, keepdims=True)).astype(np.float32)
    return idx, sc


def run(hidden_states, router_logits, w1, w2, trace=False, trace_kwargs=None):
    hs = np.asarray(hidden_states, dtype=np.float32)
    rl = np.asarray(router_logits, dtype=np.float32)
    w1 = np.asarray(w1, dtype=np.float32)
    w2 = np.asarray(w2, dtype=np.float32)
    N, D = hs.shape

    idx, sc = _route(rl)

    tok_lists = []
    for e in range(NUM_EXPERTS):
        toks, slots = np.nonzero(idx == e)
        tok_lists.append((toks, slots))
    cmax = max(len(t) for t, _ in tok_lists)

    # Full-width (N=512) matmuls stream ~5% fewer PE cycles than two ragged
    # chunks. When the capacity overhang past a 512 multiple is small, cap
    # the device capacity at the multiple and run the few overflow tokens
    # through a f32 numpy epilogue on the host (<= 64 rows per expert;
    # routing/combine already live there).
    if cmax > 512 and cmax % 512 <= 64:
        C_dev = 512 * (cmax // 512)
    else:
        C_dev = cmax
    NCH = max(1, -(-C_dev // 512))
    CH = -(-C_dev // (NCH * 2)) * 2  # chunk width, multiple of 2
    C_pad = CH * NCH

    in_maps = []
    for e in range(NUM_EXPERTS):
        toks = tok_lists[e][0][:C_pad]
        x = np.zeros((C_pad, D), np.float32)
        x[: len(toks)] = hs[toks]
        xt = x.T.reshape(KT1, P, NCH, CH).transpose(1, 0, 2, 3).astype(np_bf16)
        # [p, mt, kt, m] with the mt axis pair-interleaved: proj tile mt and
        # gate tile mt+MT1H land adjacently so pair mt needs one DMA chunk.
        w1e = w1[e].reshape(KT1, P, MT1, P).transpose(1, 2, 0, 3)[:, _W1_ORDER]
        w1e = w1e.astype(np_bf16)
        w2e = w2[e].reshape(KT2, P, MT2, P).transpose(1, 2, 0, 3).astype(np_bf16)
        in_maps.append({"xt": xt, "w1": w1e, "w2": w2e})

    nc = _get_graph(NCH, CH)

    res = None
    for attempt in range(4):
        try:
            res = run_bass_kernel_spmd(
                nc,
                in_maps,
                core_ids=list(range(NCORES)),
                trace=trace,
                **(trace_kwargs or {}),
            )
            break
        except Exception:
            if attempt == 3:
                raise
            time.sleep(15 * (attempt + 1))

    out = np.zeros((N, D), np.float32)
    for e in range(NUM_EXPERTS):
        toks, slots = tok_lists[e]
        n_dev = min(len(toks), C_pad)
        oT = np.asarray(res.results[e]["out"]).astype(np.float32)
        oT = oT.transpose(0, 2, 1, 3).reshape(HIDDEN, C_pad)
        out[toks[:n_dev]] += sc[toks[:n_dev], slots[:n_dev]][:, None] * oT[:, :n_dev].T
        if n_dev < len(toks):
            # f32 host epilogue for the few overflow tokens past capacity
            ot, osl = toks[n_dev:], slots[n_dev:]
            h = hs[ot] @ w1[e]
            proj, gate = h[:, :INTER], h[:, INTER:]
            o = (proj / (1.0 + np.exp(-proj)) * gate) @ w2[e]
            out[ot] += sc[ot, osl][:, None] * o
    return out, res


def kernel(hidden_states, router_logits, w1, w2):
    out, _ = run(hidden_states, router_logits, w1, w2)
    return out

